# revision 27
# baseline (speedup 1.0000x reference)
"""Distributed GraphSAGE (3-layer, mean aggregation) on 8 Trainium2 NeuronCores.

Strategy (ClusterGCN-style node partitioning):
  - Nodes are load-balance-packed into 128-slot "blocks" (degree-aware snake
    packing), 49 blocks per core -> each core owns 6272 node slots.
  - The gather table (h, bf16) is split into two parts: part1 = every core's
    blocks 0..24, part2 = blocks 25..48.  Each part is AllGathered as soon as
    its blocks finish, so collectives overlap compute, and each part stays
    below the int16 index range of dma_gather.
  - Each layer runs two passes over its 49 destination blocks: pass A
    aggregates part1-sourced edges (partial sums parked in SBUF), pass B
    aggregates part2-sourced edges, combines, and runs the dense epilogue.
    Pass A only depends on part1's AllGather, pass B on part2's -> the Pool
    engine (descriptor generation, the critical resource) never idles at
    layer boundaries.
  - Per block, per pass: dma_gather (custom SWDGE ucode, <=1024 descriptors
    per call, round-robin over 4 SWDGE queues, per-block-trimmed counts)
    pulls source rows (bf16); the Vector engine builds a one-hot
    (dst-slot == iota) bf16 matrix; the Tensor engine accumulates
    agg[j, f] += onehot[e, j]^T @ msgs[e, f] in PSUM = segment_sum over the
    block's 128 destination slots.
  - mean = agg * deg_inv (per-partition scale, Scalar engine), transpose on
    the Tensor engine, dense h = mean @ Wl + x @ Wr + bl (+ ReLU, f32).
Weights are replicated; the permuted x (bf16, in part layout) is replicated
as the layer-0 gather table.
"""

import sys

if "/opt/trn_rl_repo" not in sys.path:
    sys.path.insert(0, "/opt/trn_rl_repo")

import os
import numpy as np
import ml_dtypes

BF16 = ml_dtypes.bfloat16
P = 128


class Cfg:
    def __init__(self, n_nodes, n_edges, in_f, hid, out_f, n_cores, blocks_per_core):
        self.n_nodes = n_nodes
        self.n_edges = n_edges
        self.in_f = in_f
        self.hid = hid
        self.out_f = out_f
        self.out_p = 64
        self.n_cores = n_cores
        self.nb = blocks_per_core
        self.nb1 = (blocks_per_core + 1) // 2       # blocks in part1
        self.nb2 = blocks_per_core - self.nb1       # blocks in part2
        self.nblk = n_cores * blocks_per_core
        self.slots_core = blocks_per_core * P
        self.slots_tot = self.nblk * P
        self.p1_rows = n_cores * self.nb1 * P
        self.p2_rows = n_cores * self.nb2 * P
        assert self.p1_rows < 2**15 and self.p2_rows < 2**15


FULL_CFG = Cfg(n_nodes=50000, n_edges=800000, in_f=128, hid=128, out_f=47,
               n_cores=8, blocks_per_core=49)

MAXD = 1024  # dma_gather descriptor-ring limit per call


def _ru16(x):
    return (int(x) + 15) // 16 * 16


class EdgePlan:
    """Static per-(block, part) gather layout, identical across cores."""

    def __init__(self, caps):
        # caps[b][part] = padded edge count (multiple of 16)
        self.caps = caps
        nblocks = len(caps)
        self.tsub = [[(c + P - 1) // P for c in caps[b]] for b in range(nblocks)]
        self.dcol_off = []   # dstloc column offset per block
        self.icol_off = []   # wrapped-idx column offset per (block, part)
        d = 0
        i = 0
        for b in range(nblocks):
            self.dcol_off.append(d)
            offs = []
            for part in range(2):
                offs.append(i)
                i += caps[b][part] // 16
            self.icol_off.append(offs)
            d += self.tsub[b][0] + self.tsub[b][1]
        self.dcols = d
        self.icols = i

    def chunks(self, b, part):
        """Yield (t0_local, num_idxs, icol) chunks of <=MAXD descriptors."""
        cap = self.caps[b][part]
        icol = self.icol_off[b][part]
        done = 0
        while done < cap:
            n = min(MAXD, cap - done)
            yield done // P, n, icol + done // 16
            done += n


def prep_host(cfg: Cfg, x, src, dst):
    N, NBLK = cfg.n_nodes, cfg.nblk
    deg = np.bincount(dst, minlength=N).astype(np.int64)
    deg_inv = (1.0 / np.maximum(deg, 1)).astype(np.float32)

    # Degree-sorted snake deal -> balanced block loads, <=128 slots/block.
    order = np.argsort(-deg, kind="stable")
    k = np.arange(N)
    s, r = k // NBLK, k % NBLK
    blk_for_rank = np.where(s % 2 == 0, r, NBLK - 1 - r)
    blk_of_node = np.empty(N, np.int64)
    blk_of_node[order] = blk_for_rank
    slot_of_node = np.empty(N, np.int64)
    slot_of_node[order] = s
    assert slot_of_node.max() < P

    pos_of_node = blk_of_node * P + slot_of_node
    node_of_pos = np.full(cfg.slots_tot, -1, np.int64)
    node_of_pos[pos_of_node] = np.arange(N)

    # Source position in part-table coordinates
    n_core = blk_of_node // cfg.nb      # owning core
    n_bl = blk_of_node % cfg.nb         # block-local index
    n_part = (n_bl >= cfg.nb1).astype(np.int64)
    n_blp = np.where(n_part == 0, n_bl, n_bl - cfg.nb1)
    nbp = np.array([cfg.nb1, cfg.nb2])
    n_pos_p = (n_core * nbp[n_part] + n_blp) * P + slot_of_node

    # Edge keys: (dst block, src part)
    e_blk = blk_of_node[dst]
    e_part = n_part[src]
    key = e_blk * 2 + e_part
    e_order = np.argsort(key, kind="stable")
    cnt = np.bincount(key, minlength=NBLK * 2).reshape(NBLK, 2)
    # per-(local block, part) cap = max over cores, rounded to 16
    cnt_c = cnt.reshape(cfg.n_cores, cfg.nb, 2)
    caps = [[_ru16(cnt_c[:, b, part].max()) for part in range(2)]
            for b in range(cfg.nb)]
    plan = EdgePlan(caps)

    starts = np.zeros(NBLK * 2 + 1, np.int64)
    np.cumsum(cnt.reshape(-1), out=starts[1:])
    rank = np.arange(cfg.n_edges) - starts[key[e_order]]

    es, ed = src[e_order], dst[e_order]
    eb, ep = e_blk[e_order], e_part[e_order]
    e_core = eb // cfg.nb
    e_bl = eb % cfg.nb

    dstloc = np.full((cfg.n_cores, P, plan.dcols), float(P), np.float32)
    idxw = np.zeros((cfg.n_cores, 16, plan.icols), np.int16)

    dcol_off = np.array(plan.dcol_off)
    t0_of_part = np.array([[0, plan.tsub[b][0]] for b in range(cfg.nb)])
    icol_off = np.array(plan.icol_off)

    # dstloc[core, p, dcol_off[b] + t0_of_part + rank//P] (col), row rank%P
    dcol = dcol_off[e_bl] + t0_of_part[e_bl, ep] + rank // P
    dstloc[e_core, rank % P, dcol] = slot_of_node[ed].astype(np.float32)
    # wrapped idx: within (b, part) region, idx i -> [i%16, off + i//16]
    icol = icol_off[e_bl, ep] + rank // 16
    idxw[e_core, rank % 16, icol] = n_pos_p[es].astype(np.int16)

    dstloc = dstloc.astype(BF16)
    idx_arr = np.ascontiguousarray(np.tile(idxw, (1, 8, 1)))

    dinv_slot = np.ones((cfg.n_cores, P, cfg.nb), np.float32)
    vpos = np.nonzero(node_of_pos >= 0)[0]
    vcore = vpos // cfg.slots_core
    vloc = vpos % cfg.slots_core
    dinv_slot[vcore, vloc % P, vloc // P] = deg_inv[node_of_pos[vpos]]

    # x in part layout (bf16 gather tables) + per-core transposed slice (f32)
    x_perm = np.zeros((cfg.slots_tot, cfg.in_f), np.float32)
    x_perm[pos_of_node] = x
    xp1 = np.zeros((cfg.p1_rows, cfg.in_f), np.float32)
    xp2 = np.zeros((cfg.p2_rows, cfg.in_f), np.float32)
    m1 = n_part[np.arange(N)] == 0
    xp1[n_pos_p[m1]] = x[m1]
    xp2[n_pos_p[~m1]] = x[~m1]
    xT = np.ascontiguousarray(
        x_perm.reshape(cfg.n_cores, cfg.slots_core, cfg.in_f).transpose(0, 2, 1))

    return dict(plan=plan, idx=idx_arr, dstloc=dstloc, dinv=dinv_slot,
                xp1=xp1.astype(BF16), xp2=xp2.astype(BF16), xT=xT,
                node_of_pos=node_of_pos)


# --------------------------------------------------------------------------
# Device program
# --------------------------------------------------------------------------

def build_program(cfg: Cfg, plan: EdgePlan):
    import concourse.bacc as bacc
    import concourse.tile as tile
    from concourse import bass, mybir
    from concourse.library_config import mlp

    f32 = mybir.dt.float32
    bf16 = mybir.dt.bfloat16
    i16 = mybir.dt.int16
    AF = mybir.ActivationFunctionType
    OP = mybir.AluOpType

    NB, NB1, S = cfg.nb, cfg.nb1, cfg.slots_core
    HID, OUTP = cfg.hid, cfg.out_p
    NQ = 4
    TMAX = max(max(plan.tsub[b]) for b in range(NB))

    nc = bacc.Bacc("TRN2", target_bir_lowering=False, debug=False,
                   enable_asserts=False, num_devices=cfg.n_cores,
                   num_swdge_queues=NQ)

    xp1_d = nc.dram_tensor("xp1", [cfg.p1_rows, cfg.in_f], bf16,
                           kind="ExternalInput").ap()
    xp2_d = nc.dram_tensor("xp2", [cfg.p2_rows, cfg.in_f], bf16,
                           kind="ExternalInput").ap()
    xtloc = nc.dram_tensor("xtloc", [cfg.in_f, S], f32,
                           kind="ExternalInput").ap()
    idx_d = nc.dram_tensor("idx", [P, plan.icols], i16,
                           kind="ExternalInput").ap()
    dstloc_d = nc.dram_tensor("dstloc", [P, plan.dcols], bf16,
                              kind="ExternalInput").ap()
    dinv_d = nc.dram_tensor("dinv", [P, NB], f32, kind="ExternalInput").ap()
    w_d = {}
    for name, shp, dt_ in [("wl0", [cfg.in_f, HID], f32),
                           ("wr0", [cfg.in_f, HID], f32),
                           ("wl1", [HID, HID], f32), ("wr1", [HID, HID], f32),
                           ("wl2", [HID, OUTP], f32), ("wr2", [HID, OUTP], f32),
                           ("bl0", [1, HID], f32), ("bl1", [1, HID], f32),
                           ("bl2", [1, OUTP], f32),
                           ("iota", [P, P], bf16), ("ident", [P, P], f32),
                           ("identb", [P, P], bf16), ("ones", [1, P], f32)]:
        w_d[name] = nc.dram_tensor(name, shp, dt_, kind="ExternalInput").ap()
    out_d = nc.dram_tensor("out", [S, OUTP], f32, kind="ExternalOutput").ap()

    qload = [0] * 4  # per-queue accumulated descriptor count

    with tile.TileContext(nc) as tc:
        with tc.tile_critical():
            nc.gpsimd.load_library(mlp)
        with (
            tc.tile_pool(name="const", bufs=1) as cp,
            tc.tile_pool(name="hT", bufs=2) as hTp,
            tc.tile_pool(name="msgs", bufs=4) as mp,
            tc.tile_pool(name="oh", bufs=3) as ohp,
            tc.tile_pool(name="sb", bufs=3) as sbp,
            tc.tile_pool(name="pagg", bufs=2, space="PSUM") as pagg,
            tc.tile_pool(name="ptr", bufs=2, space="PSUM") as ptr,
            tc.tile_pool(name="pd", bufs=2, space="PSUM") as pd,
            tc.tile_pool(name="dram", bufs=2, space="DRAM") as dp,
        ):
            def load_const(ap_, shp, dtype=f32, tag=None):
                t = cp.tile(shp, dtype, tag=tag or ap_.tensor.name)
                nc.sync.dma_start(out=t[:], in_=ap_)
                return t

            idx_sb = load_const(idx_d, [P, plan.icols], i16)
            dst_sb = load_const(dstloc_d, [P, plan.dcols], bf16)
            dinv_sb = load_const(dinv_d, [P, NB])
            iota_sb = load_const(w_d["iota"], [P, P], bf16)
            ident_sb = load_const(w_d["ident"], [P, P])
            identb_sb = load_const(w_d["identb"], [P, P], bf16)
            ones_sb = load_const(w_d["ones"], [1, P])
            wsb = {k: load_const(w_d[k], list(w_d[k].shape))
                   for k in ("wl0", "wr0", "wl1", "wr1", "wl2", "wr2",
                             "bl0", "bl1", "bl2")}

            hT_x = hTp.tile([P, S], f32, tag="hT", name="hT_x")
            nc.sync.dma_start(out=hT_x[:], in_=xtloc)

            iota_b = iota_sb[:].rearrange("p (a c) -> p a c", a=1)

            def gather_part(table_ap, b, part):
                tsub = plan.tsub[b][part]
                if tsub == 0:
                    return None
                msgs = mp.tile([P, TMAX * HID], bf16, tag="msgs")
                cap = plan.caps[b][part]
                if cap % P:
                    # zero the final subtile before gathering into it: rows
                    # the gather won't write must stay finite (0 * stale-NaN
                    # would poison the aggregation)
                    nc.vector.memset(
                        msgs[:, (tsub - 1) * HID:tsub * HID], 0)
                for t0, nidx, icol in plan.chunks(b, part):
                    nt = (nidx + P - 1) // P
                    q = min(range(NQ), key=lambda i: qload[i])
                    qload[q] += nidx
                    nc.gpsimd.dma_gather(
                        out_ap=msgs[:, t0 * HID:(t0 + nt) * HID].rearrange(
                            "p (t c) -> p t c", c=HID),
                        in_ap=table_ap,
                        idxs_ap=idx_sb[:, icol:icol + nidx // 16],
                        num_idxs=nidx,
                        num_idxs_reg=nidx,
                        elem_size=HID,
                        queue_num=q,
                    )
                return msgs

            def agg_matmuls(msgs, b, part, agg_ps):
                tsub = plan.tsub[b][part]
                dcol = plan.dcol_off[b] + (0 if part == 0 else plan.tsub[b][0])
                O = ohp.tile([P, TMAX * P], bf16, tag="oh")
                nc.vector.tensor_tensor(
                    out=O[:, :tsub * P].rearrange("p (t c) -> p t c", t=tsub),
                    in0=dst_sb[:, dcol:dcol + tsub].to_broadcast([P, tsub, P]),
                    in1=iota_b.to_broadcast([P, tsub, P]),
                    op=OP.is_equal,
                )
                for t in range(tsub):
                    nc.tensor.matmul(
                        out=agg_ps[:],
                        lhsT=O[:, t * P:(t + 1) * P],
                        rhs=msgs[:, t * HID:(t + 1) * HID],
                        start=(t == 0), stop=(t == tsub - 1),
                    )

            def pass_a(table_ap, aggbuf):
                for b in range(NB):
                    msgs = gather_part(table_ap, b, 0)
                    agg_ps = pagg.tile([P, HID], f32, tag="agg")
                    if msgs is None:
                        nc.vector.memset(aggbuf[:, b * P:(b + 1) * P], 0)
                        continue
                    agg_matmuls(msgs, b, 0, agg_ps)
                    nc.vector.tensor_copy(out=aggbuf[:, b * P:(b + 1) * P],
                                          in_=agg_ps[:])

            def pass_b(li, table_ap, aggbuf, Wl, Wr, bl, hT_prev, hT_cur,
                       bounce1, bounce2):
                K = OUTP if li == 2 else HID
                for b in range(NB):
                    msgs = gather_part(table_ap, b, 1)
                    mean_pre = sbp.tile([P, HID], f32, tag="mean_pre")
                    if msgs is not None:
                        agg_ps = pagg.tile([P, HID], f32, tag="agg")
                        agg_matmuls(msgs, b, 1, agg_ps)
                        nc.vector.tensor_tensor(
                            out=mean_pre[:], in0=aggbuf[:, b * P:(b + 1) * P],
                            in1=agg_ps[:], op=OP.add)
                    else:
                        nc.vector.tensor_copy(
                            out=mean_pre[:], in_=aggbuf[:, b * P:(b + 1) * P])
                    mean_sb = sbp.tile([P, HID], f32, tag="mean")
                    nc.scalar.activation(mean_sb[:], mean_pre[:], AF.Copy,
                                         scale=dinv_sb[:, b:b + 1])
                    ptr_t = ptr.tile([P, P], f32, tag="tr")
                    nc.tensor.transpose(ptr_t[:], mean_sb[:], ident_sb[:])
                    meanT = sbp.tile([P, P], f32, tag="meanT")
                    nc.vector.tensor_copy(out=meanT[:], in_=ptr_t[:])

                    dens = pd.tile([P, K], f32, tag="dense")
                    nc.tensor.matmul(out=dens[:], lhsT=meanT[:], rhs=Wl[:],
                                     start=True, stop=False)
                    nc.tensor.matmul(out=dens[:],
                                     lhsT=hT_prev[:, b * P:(b + 1) * P],
                                     rhs=Wr[:], start=False, stop=False)
                    nc.tensor.matmul(out=dens[:], lhsT=ones_sb[:], rhs=bl[:],
                                     start=False, stop=True)
                    if li < 2:
                        h_b = sbp.tile([P, HID], bf16, tag="h_b")
                        nc.scalar.activation(h_b[:], dens[:], AF.Relu)
                        if b < NB1:
                            nc.sync.dma_start(
                                out=bounce1[b * P:(b + 1) * P, :], in_=h_b[:])
                        else:
                            bb = b - NB1
                            nc.sync.dma_start(
                                out=bounce2[bb * P:(bb + 1) * P, :], in_=h_b[:])
                        ptr_t2 = ptr.tile([P, P], bf16, tag="trb", bufs=1)
                        nc.tensor.transpose(ptr_t2[:], h_b[:], identb_sb[:])
                        nc.vector.tensor_copy(out=hT_cur[:, b * P:(b + 1) * P],
                                              in_=ptr_t2[:])
                    else:
                        out_b = sbp.tile([P, OUTP], f32, tag="out_b")
                        nc.scalar.activation(out_b[:], dens[:], AF.Copy)
                        nc.sync.dma_start(out=out_d[b * P:(b + 1) * P, :],
                                          in_=out_b[:])

            def allgather(bounce, full):
                nc.gpsimd.collective_compute(
                    "AllGather", OP.bypass,
                    replica_groups=[list(range(cfg.n_cores))],
                    ins=[bounce.opt()], outs=[full.opt()],
                )

            # DRAM staging (double buffered across layers)
            tb1 = [dp.tile([cfg.p1_rows, HID], bf16, tag="t1",
                           addr_space="Shared", name=f"t1_{i}") for i in range(2)]
            tb2 = [dp.tile([cfg.p2_rows, HID], bf16, tag="t2",
                           addr_space="Shared", name=f"t2_{i}") for i in range(2)]
            bn1 = [dp.tile([cfg.nb1 * P, HID], bf16, tag="b1", name=f"b1_{i}")
                   for i in range(2)]
            bn2 = [dp.tile([cfg.nb2 * P, HID], bf16, tag="b2", name=f"b2_{i}")
                   for i in range(2)]

            hT = [hTp.tile([P, S], f32, tag="hT", name=f"hT{i}")
                  for i in range(2)]
            ab = [hTp.tile([P, S], f32, tag="aggbuf", name=f"ab{i}")
                  for i in range(2)]

            # layer 0 (tables = x parts, already resident)
            pass_a(xp1_d, ab[0])
            pass_b(0, xp2_d, ab[0], wsb["wl0"], wsb["wr0"], wsb["bl0"],
                   hT_x, hT[0], bn1[0][:], bn2[0][:])
            allgather(bn1[0], tb1[0])
            allgather(bn2[0], tb2[0])

            # layer 1
            pass_a(tb1[0][:], ab[1])
            pass_b(1, tb2[0][:], ab[1], wsb["wl1"], wsb["wr1"], wsb["bl1"],
                   hT[0], hT[1], bn1[1][:], bn2[1][:])
            allgather(bn1[1], tb1[1])
            allgather(bn2[1], tb2[1])

            # layer 2
            pass_a(tb1[1][:], ab[0])
            pass_b(2, tb2[1][:], ab[0], wsb["wl2"], wsb["wr2"], wsb["bl2"],
                   hT[1], None, None, None)

    nc.compile()
    return nc


# --------------------------------------------------------------------------
# Entry point
# --------------------------------------------------------------------------

def _make_in_maps(cfg: Cfg, host, weights):
    iota = np.broadcast_to(np.arange(P, dtype=np.float32), (P, P)).astype(BF16)
    ident = np.eye(P, dtype=np.float32)
    ones = np.ones((1, P), np.float32)
    maps = []
    for i in range(cfg.n_cores):
        m = dict(
            xp1=host["xp1"], xp2=host["xp2"],
            xtloc=host["xT"][i],
            idx=host["idx"][i],
            dstloc=host["dstloc"][i],
            dinv=host["dinv"][i],
            iota=iota, ident=ident, identb=ident.astype(BF16), ones=ones,
        )
        m.update(weights)
        maps.append(m)
    return maps


def _pad_w(w, outp):
    w = np.asarray(w, np.float32)
    if w.shape[-1] < outp:
        pad = np.zeros(w.shape[:-1] + (outp - w.shape[-1],), np.float32)
        w = np.concatenate([w, pad], axis=-1)
    return w


def _ensure_ntff_hook():
    import types

    try:
        from antenv.axon_hooks import get_axon_ntff_profile_hook  # noqa: F401
        return
    except ImportError:
        pass
    try:
        import antenv
        from trn_agent_boot.trn_boot import _ntff_profile_via_ctypes
    except ImportError:
        return
    hook = _ntff_profile_via_ctypes("/opt/axon/libaxon_pjrt.so")
    mod = types.ModuleType("antenv.axon_hooks")
    mod.get_axon_ntff_profile_hook = lambda: hook
    mod.set_axon_ntff_profile_hook = lambda h: None
    sys.modules["antenv.axon_hooks"] = mod
    antenv.axon_hooks = mod


def run(cfg: Cfg, inputs, trace=False):
    import concourse.bass_utils as bu
    from concourse.bass_utils import run_bass_kernel_spmd

    if trace:
        _ensure_ntff_hook()
        bu.upload_artifacts = lambda d: str(d)

    x = np.asarray(inputs["x"], np.float32)
    ei = np.asarray(inputs["edge_index"])
    src = ei[0].astype(np.int64)
    dst = ei[1].astype(np.int64)

    host = prep_host(cfg, x, src, dst)
    weights = dict(
        wl0=np.asarray(inputs["Wl0"], np.float32),
        wr0=np.asarray(inputs["Wr0"], np.float32),
        wl1=np.asarray(inputs["Wl1"], np.float32),
        wr1=np.asarray(inputs["Wr1"], np.float32),
        wl2=_pad_w(inputs["Wl2"], cfg.out_p),
        wr2=_pad_w(inputs["Wr2"], cfg.out_p),
        bl0=np.asarray(inputs["bl0"], np.float32).reshape(1, -1),
        bl1=np.asarray(inputs["bl1"], np.float32).reshape(1, -1),
        bl2=_pad_w(np.asarray(inputs["bl2"], np.float32).reshape(1, -1),
                   cfg.out_p),
    )

    nc = build_program(cfg, host["plan"])
    in_maps = _make_in_maps(cfg, host, weights)
    res = run_bass_kernel_spmd(nc, in_maps, core_ids=list(range(cfg.n_cores)),
                               trace=trace)

    out_full = np.empty((cfg.n_nodes, cfg.out_f), np.float32)
    node_of_pos = host["node_of_pos"]
    for i in range(cfg.n_cores):
        o = res.results[i]["out"]
        pos = np.arange(i * cfg.slots_core, (i + 1) * cfg.slots_core)
        nodes = node_of_pos[pos]
        valid = nodes >= 0
        out_full[nodes[valid]] = o[valid][:, :cfg.out_f]
    return out_full, res


def kernel(**inputs) -> np.ndarray:
    trace = os.environ.get("GNN_TRACE", "0") == "1"
    out, _ = run(FULL_CFG, inputs, trace=trace)
    return out


# revision 29
# speedup vs baseline: 1.0488x; 1.0488x over previous
"""Distributed GraphSAGE (3-layer, mean aggregation) on 8 Trainium2 NeuronCores.

Strategy (ClusterGCN-style node partitioning):
  - Nodes are load-balance-packed into 128-slot "blocks" (degree-aware snake
    packing), 49 blocks per core -> each core owns 6272 node slots.
  - The gather table (h, bf16) is split into two parts: part1 = every core's
    blocks 0..24, part2 = blocks 25..48.  Each part is AllGathered as soon as
    its blocks finish, so collectives overlap compute, and each part stays
    below the int16 index range of dma_gather.
  - Each layer runs two passes over its 49 destination blocks: pass A
    aggregates part1-sourced edges (partial sums parked in SBUF), pass B
    aggregates part2-sourced edges, combines, and runs the dense epilogue.
    Pass A only depends on part1's AllGather, pass B on part2's -> the Pool
    engine (descriptor generation, the critical resource) never idles at
    layer boundaries.
  - Per block, per pass: dma_gather (custom SWDGE ucode, <=1024 descriptors
    per call, round-robin over 4 SWDGE queues, per-block-trimmed counts)
    pulls source rows (bf16); the Vector engine builds a one-hot
    (dst-slot == iota) bf16 matrix; the Tensor engine accumulates
    agg[j, f] += onehot[e, j]^T @ msgs[e, f] in PSUM = segment_sum over the
    block's 128 destination slots.
  - mean = agg * deg_inv (per-partition scale, Scalar engine), transpose on
    the Tensor engine, dense h = mean @ Wl + x @ Wr + bl (+ ReLU, f32).
Weights are replicated; the permuted x (bf16, in part layout) is replicated
as the layer-0 gather table.
"""

import sys

if "/opt/trn_rl_repo" not in sys.path:
    sys.path.insert(0, "/opt/trn_rl_repo")

import os
import numpy as np
import ml_dtypes

BF16 = ml_dtypes.bfloat16
P = 128


class Cfg:
    def __init__(self, n_nodes, n_edges, in_f, hid, out_f, n_cores, blocks_per_core):
        self.n_nodes = n_nodes
        self.n_edges = n_edges
        self.in_f = in_f
        self.hid = hid
        self.out_f = out_f
        self.out_p = 64
        self.n_cores = n_cores
        self.nb = blocks_per_core
        self.nb1 = (blocks_per_core + 1) // 2       # blocks in part1
        self.nb2 = blocks_per_core - self.nb1       # blocks in part2
        self.nblk = n_cores * blocks_per_core
        self.slots_core = blocks_per_core * P
        self.slots_tot = self.nblk * P
        self.p1_rows = n_cores * self.nb1 * P
        self.p2_rows = n_cores * self.nb2 * P
        assert self.p1_rows < 2**15 and self.p2_rows < 2**15


FULL_CFG = Cfg(n_nodes=50000, n_edges=800000, in_f=128, hid=128, out_f=47,
               n_cores=8, blocks_per_core=49)

MAXD = 1024  # dma_gather descriptor-ring limit per call


def _ru16(x):
    return (int(x) + 15) // 16 * 16


class EdgePlan:
    """Static per-(block, part) gather layout, identical across cores."""

    def __init__(self, caps):
        # caps[b][part] = padded edge count (multiple of 16)
        self.caps = caps
        nblocks = len(caps)
        self.tsub = [[(c + P - 1) // P for c in caps[b]] for b in range(nblocks)]
        self.dcol_off = []   # dstloc column offset per block
        self.icol_off = []   # wrapped-idx column offset per (block, part)
        d = 0
        i = 0
        for b in range(nblocks):
            self.dcol_off.append(d)
            offs = []
            for part in range(2):
                offs.append(i)
                i += caps[b][part] // 16
            self.icol_off.append(offs)
            d += self.tsub[b][0] + self.tsub[b][1]
        self.dcols = d
        self.icols = i

    def chunks(self, b, part):
        """Yield (t0_local, num_idxs, icol) chunks of <=MAXD descriptors."""
        cap = self.caps[b][part]
        icol = self.icol_off[b][part]
        done = 0
        while done < cap:
            n = min(MAXD, cap - done)
            yield done // P, n, icol + done // 16
            done += n


def prep_host(cfg: Cfg, x, src, dst):
    N, NBLK = cfg.n_nodes, cfg.nblk
    deg = np.bincount(dst, minlength=N).astype(np.int64)
    deg_inv = (1.0 / np.maximum(deg, 1)).astype(np.float32)

    # Degree-sorted snake deal -> balanced block loads, <=128 slots/block.
    order = np.argsort(-deg, kind="stable")
    k = np.arange(N)
    s, r = k // NBLK, k % NBLK
    blk_for_rank = np.where(s % 2 == 0, r, NBLK - 1 - r)
    blk_of_node = np.empty(N, np.int64)
    blk_of_node[order] = blk_for_rank
    slot_of_node = np.empty(N, np.int64)
    slot_of_node[order] = s
    assert slot_of_node.max() < P

    pos_of_node = blk_of_node * P + slot_of_node
    node_of_pos = np.full(cfg.slots_tot, -1, np.int64)
    node_of_pos[pos_of_node] = np.arange(N)

    # Source position in part-table coordinates
    n_core = blk_of_node // cfg.nb      # owning core
    n_bl = blk_of_node % cfg.nb         # block-local index
    n_part = (n_bl >= cfg.nb1).astype(np.int64)
    n_blp = np.where(n_part == 0, n_bl, n_bl - cfg.nb1)
    nbp = np.array([cfg.nb1, cfg.nb2])
    n_pos_p = (n_core * nbp[n_part] + n_blp) * P + slot_of_node

    # Edge keys: (dst block, src part)
    e_blk = blk_of_node[dst]
    e_part = n_part[src]
    key = e_blk * 2 + e_part
    e_order = np.argsort(key, kind="stable")
    cnt = np.bincount(key, minlength=NBLK * 2).reshape(NBLK, 2)
    # per-(local block, part) cap = max over cores, rounded to 16
    cnt_c = cnt.reshape(cfg.n_cores, cfg.nb, 2)
    caps = [[_ru16(cnt_c[:, b, part].max()) for part in range(2)]
            for b in range(cfg.nb)]
    plan = EdgePlan(caps)

    starts = np.zeros(NBLK * 2 + 1, np.int64)
    np.cumsum(cnt.reshape(-1), out=starts[1:])
    rank = np.arange(cfg.n_edges) - starts[key[e_order]]

    es, ed = src[e_order], dst[e_order]
    eb, ep = e_blk[e_order], e_part[e_order]
    e_core = eb // cfg.nb
    e_bl = eb % cfg.nb

    dstloc = np.full((cfg.n_cores, P, plan.dcols), float(P), np.float32)
    idxw = np.zeros((cfg.n_cores, 16, plan.icols), np.int16)

    dcol_off = np.array(plan.dcol_off)
    t0_of_part = np.array([[0, plan.tsub[b][0]] for b in range(cfg.nb)])
    icol_off = np.array(plan.icol_off)

    # dstloc[core, p, dcol_off[b] + t0_of_part + rank//P] (col), row rank%P
    dcol = dcol_off[e_bl] + t0_of_part[e_bl, ep] + rank // P
    dstloc[e_core, rank % P, dcol] = slot_of_node[ed].astype(np.float32)
    # wrapped idx: within (b, part) region, idx i -> [i%16, off + i//16]
    icol = icol_off[e_bl, ep] + rank // 16
    idxw[e_core, rank % 16, icol] = n_pos_p[es].astype(np.int16)

    dstloc = dstloc.astype(BF16)
    idx_arr = np.ascontiguousarray(np.tile(idxw, (1, 8, 1)))

    dinv_slot = np.ones((cfg.n_cores, P, cfg.nb), np.float32)
    vpos = np.nonzero(node_of_pos >= 0)[0]
    vcore = vpos // cfg.slots_core
    vloc = vpos % cfg.slots_core
    dinv_slot[vcore, vloc % P, vloc // P] = deg_inv[node_of_pos[vpos]]

    # x in part layout (bf16 gather tables) + per-core transposed slice (f32)
    x_perm = np.zeros((cfg.slots_tot, cfg.in_f), np.float32)
    x_perm[pos_of_node] = x
    xp1 = np.zeros((cfg.p1_rows, cfg.in_f), np.float32)
    xp2 = np.zeros((cfg.p2_rows, cfg.in_f), np.float32)
    m1 = n_part[np.arange(N)] == 0
    xp1[n_pos_p[m1]] = x[m1]
    xp2[n_pos_p[~m1]] = x[~m1]
    xT = np.ascontiguousarray(
        x_perm.reshape(cfg.n_cores, cfg.slots_core, cfg.in_f).transpose(0, 2, 1))

    return dict(plan=plan, idx=idx_arr, dstloc=dstloc, dinv=dinv_slot,
                xp1=xp1.astype(BF16), xp2=xp2.astype(BF16), xT=xT,
                node_of_pos=node_of_pos)


# --------------------------------------------------------------------------
# Device program
# --------------------------------------------------------------------------

def build_program(cfg: Cfg, plan: EdgePlan):
    import concourse.bacc as bacc
    import concourse.tile as tile
    from concourse import bass, mybir
    from concourse.library_config import mlp

    f32 = mybir.dt.float32
    bf16 = mybir.dt.bfloat16
    i16 = mybir.dt.int16
    AF = mybir.ActivationFunctionType
    OP = mybir.AluOpType

    NB, NB1, S = cfg.nb, cfg.nb1, cfg.slots_core
    HID, OUTP = cfg.hid, cfg.out_p
    NQ = 4
    TMAX = max(max(plan.tsub[b]) for b in range(NB))

    nc = bacc.Bacc("TRN2", target_bir_lowering=False, debug=False,
                   enable_asserts=False, num_devices=cfg.n_cores,
                   num_swdge_queues=NQ)

    xp1_d = nc.dram_tensor("xp1", [cfg.p1_rows, cfg.in_f], bf16,
                           kind="ExternalInput").ap()
    xp2_d = nc.dram_tensor("xp2", [cfg.p2_rows, cfg.in_f], bf16,
                           kind="ExternalInput").ap()
    xtloc = nc.dram_tensor("xtloc", [cfg.in_f, S], f32,
                           kind="ExternalInput").ap()
    idx_d = nc.dram_tensor("idx", [P, plan.icols], i16,
                           kind="ExternalInput").ap()
    dstloc_d = nc.dram_tensor("dstloc", [P, plan.dcols], bf16,
                              kind="ExternalInput").ap()
    dinv_d = nc.dram_tensor("dinv", [P, NB], f32, kind="ExternalInput").ap()
    w_d = {}
    for name, shp, dt_ in [("wl0", [cfg.in_f, HID], f32),
                           ("wr0", [cfg.in_f, HID], f32),
                           ("wl1", [HID, HID], f32), ("wr1", [HID, HID], f32),
                           ("wl2", [HID, OUTP], f32), ("wr2", [HID, OUTP], f32),
                           ("bl0", [1, HID], f32), ("bl1", [1, HID], f32),
                           ("bl2", [1, OUTP], f32),
                           ("iota", [P, P], bf16), ("ident", [P, P], f32),
                           ("identb", [P, P], bf16), ("ones", [1, P], f32)]:
        w_d[name] = nc.dram_tensor(name, shp, dt_, kind="ExternalInput").ap()
    out_d = nc.dram_tensor("out", [S, OUTP], f32, kind="ExternalOutput").ap()

    qctr = [0]

    with tile.TileContext(nc) as tc:
        with tc.tile_critical():
            nc.gpsimd.load_library(mlp)
        with (
            tc.tile_pool(name="const", bufs=1) as cp,
            tc.tile_pool(name="hT", bufs=2) as hTp,
            tc.tile_pool(name="msgs", bufs=4) as mp,
            tc.tile_pool(name="oh", bufs=3) as ohp,
            tc.tile_pool(name="sb", bufs=3) as sbp,
            tc.tile_pool(name="pagg", bufs=2, space="PSUM") as pagg,
            tc.tile_pool(name="ptr", bufs=2, space="PSUM") as ptr,
            tc.tile_pool(name="pd", bufs=2, space="PSUM") as pd,
            tc.tile_pool(name="dram", bufs=2, space="DRAM") as dp,
        ):
            def load_const(ap_, shp, dtype=f32, tag=None):
                t = cp.tile(shp, dtype, tag=tag or ap_.tensor.name)
                nc.sync.dma_start(out=t[:], in_=ap_)
                return t

            idx_sb = load_const(idx_d, [P, plan.icols], i16)
            dst_sb = load_const(dstloc_d, [P, plan.dcols], bf16)
            dinv_sb = load_const(dinv_d, [P, NB])
            iota_sb = load_const(w_d["iota"], [P, P], bf16)
            ident_sb = load_const(w_d["ident"], [P, P])
            identb_sb = load_const(w_d["identb"], [P, P], bf16)
            ones_sb = load_const(w_d["ones"], [1, P])
            wsb = {k: load_const(w_d[k], list(w_d[k].shape))
                   for k in ("wl0", "wr0", "wl1", "wr1", "wl2", "wr2",
                             "bl0", "bl1", "bl2")}

            hT_x = hTp.tile([P, S], f32, tag="hT", name="hT_x")
            nc.sync.dma_start(out=hT_x[:], in_=xtloc)

            iota_b = iota_sb[:].rearrange("p (a c) -> p a c", a=1)

            def gather_part(table_ap, b, part):
                tsub = plan.tsub[b][part]
                if tsub == 0:
                    return None
                msgs = mp.tile([P, TMAX * HID], bf16, tag="msgs")
                cap = plan.caps[b][part]
                if cap % P:
                    # zero the final subtile before gathering into it: rows
                    # the gather won't write must stay finite (0 * stale-NaN
                    # would poison the aggregation)
                    nc.vector.memset(
                        msgs[:, (tsub - 1) * HID:tsub * HID], 0)
                for t0, nidx, icol in plan.chunks(b, part):
                    nt = (nidx + P - 1) // P
                    nc.gpsimd.dma_gather(
                        out_ap=msgs[:, t0 * HID:(t0 + nt) * HID].rearrange(
                            "p (t c) -> p t c", c=HID),
                        in_ap=table_ap,
                        idxs_ap=idx_sb[:, icol:icol + nidx // 16],
                        num_idxs=nidx,
                        num_idxs_reg=nidx,
                        elem_size=HID,
                        queue_num=qctr[0] % NQ,
                    )
                    qctr[0] += 1
                return msgs

            def agg_matmuls(msgs, b, part, agg_ps):
                tsub = plan.tsub[b][part]
                dcol = plan.dcol_off[b] + (0 if part == 0 else plan.tsub[b][0])
                O = ohp.tile([P, TMAX * P], bf16, tag="oh")
                nc.vector.tensor_tensor(
                    out=O[:, :tsub * P].rearrange("p (t c) -> p t c", t=tsub),
                    in0=dst_sb[:, dcol:dcol + tsub].to_broadcast([P, tsub, P]),
                    in1=iota_b.to_broadcast([P, tsub, P]),
                    op=OP.is_equal,
                )
                for t in range(tsub):
                    nc.tensor.matmul(
                        out=agg_ps[:],
                        lhsT=O[:, t * P:(t + 1) * P],
                        rhs=msgs[:, t * HID:(t + 1) * HID],
                        start=(t == 0), stop=(t == tsub - 1),
                    )

            def pass_a(table_ap, aggbuf):
                for b in range(NB):
                    msgs = gather_part(table_ap, b, 0)
                    agg_ps = pagg.tile([P, HID], f32, tag="agg")
                    if msgs is None:
                        nc.vector.memset(aggbuf[:, b * P:(b + 1) * P], 0)
                        continue
                    agg_matmuls(msgs, b, 0, agg_ps)
                    nc.vector.tensor_copy(out=aggbuf[:, b * P:(b + 1) * P],
                                          in_=agg_ps[:])

            def pass_b(li, table_ap, aggbuf, Wl, Wr, bl, hT_prev, hT_cur,
                       bounce1, bounce2):
                K = OUTP if li == 2 else HID
                for b in range(NB):
                    msgs = gather_part(table_ap, b, 1)
                    mean_pre = sbp.tile([P, HID], f32, tag="mean_pre")
                    if msgs is not None:
                        agg_ps = pagg.tile([P, HID], f32, tag="agg")
                        agg_matmuls(msgs, b, 1, agg_ps)
                        nc.vector.tensor_tensor(
                            out=mean_pre[:], in0=aggbuf[:, b * P:(b + 1) * P],
                            in1=agg_ps[:], op=OP.add)
                    else:
                        nc.vector.tensor_copy(
                            out=mean_pre[:], in_=aggbuf[:, b * P:(b + 1) * P])
                    mean_sb = sbp.tile([P, HID], f32, tag="mean")
                    nc.scalar.activation(mean_sb[:], mean_pre[:], AF.Copy,
                                         scale=dinv_sb[:, b:b + 1])
                    ptr_t = ptr.tile([P, P], f32, tag="tr")
                    nc.tensor.transpose(ptr_t[:], mean_sb[:], ident_sb[:])
                    meanT = sbp.tile([P, P], f32, tag="meanT")
                    nc.vector.tensor_copy(out=meanT[:], in_=ptr_t[:])

                    dens = pd.tile([P, K], f32, tag="dense")
                    nc.tensor.matmul(out=dens[:], lhsT=meanT[:], rhs=Wl[:],
                                     start=True, stop=False)
                    nc.tensor.matmul(out=dens[:],
                                     lhsT=hT_prev[:, b * P:(b + 1) * P],
                                     rhs=Wr[:], start=False, stop=False)
                    nc.tensor.matmul(out=dens[:], lhsT=ones_sb[:], rhs=bl[:],
                                     start=False, stop=True)
                    if li < 2:
                        h_b = sbp.tile([P, HID], bf16, tag="h_b")
                        nc.scalar.activation(h_b[:], dens[:], AF.Relu)
                        if b < NB1:
                            nc.sync.dma_start(
                                out=bounce1[b * P:(b + 1) * P, :], in_=h_b[:])
                        else:
                            bb = b - NB1
                            nc.sync.dma_start(
                                out=bounce2[bb * P:(bb + 1) * P, :], in_=h_b[:])
                        ptr_t2 = ptr.tile([P, P], bf16, tag="trb", bufs=1)
                        nc.tensor.transpose(ptr_t2[:], h_b[:], identb_sb[:])
                        nc.vector.tensor_copy(out=hT_cur[:, b * P:(b + 1) * P],
                                              in_=ptr_t2[:])
                    else:
                        out_b = sbp.tile([P, OUTP], f32, tag="out_b")
                        nc.scalar.activation(out_b[:], dens[:], AF.Copy)
                        nc.sync.dma_start(out=out_d[b * P:(b + 1) * P, :],
                                          in_=out_b[:])

            def allgather(bounce, full):
                nc.gpsimd.collective_compute(
                    "AllGather", OP.bypass,
                    replica_groups=[list(range(cfg.n_cores))],
                    ins=[bounce.opt()], outs=[full.opt()],
                )

            # DRAM staging (double buffered across layers)
            tb1 = [dp.tile([cfg.p1_rows, HID], bf16, tag="t1",
                           addr_space="Shared", name=f"t1_{i}") for i in range(2)]
            tb2 = [dp.tile([cfg.p2_rows, HID], bf16, tag="t2",
                           addr_space="Shared", name=f"t2_{i}") for i in range(2)]
            bn1 = [dp.tile([cfg.nb1 * P, HID], bf16, tag="b1", name=f"b1_{i}")
                   for i in range(2)]
            bn2 = [dp.tile([cfg.nb2 * P, HID], bf16, tag="b2", name=f"b2_{i}")
                   for i in range(2)]

            hT = [hTp.tile([P, S], f32, tag="hT", name=f"hT{i}")
                  for i in range(2)]
            ab = [hTp.tile([P, S], f32, tag="aggbuf", name=f"ab{i}")
                  for i in range(2)]

            # layer 0 (tables = x parts, already resident)
            pass_a(xp1_d, ab[0])
            pass_b(0, xp2_d, ab[0], wsb["wl0"], wsb["wr0"], wsb["bl0"],
                   hT_x, hT[0], bn1[0][:], bn2[0][:])
            allgather(bn1[0], tb1[0])
            allgather(bn2[0], tb2[0])

            # layer 1
            pass_a(tb1[0][:], ab[1])
            pass_b(1, tb2[0][:], ab[1], wsb["wl1"], wsb["wr1"], wsb["bl1"],
                   hT[0], hT[1], bn1[1][:], bn2[1][:])
            allgather(bn1[1], tb1[1])
            allgather(bn2[1], tb2[1])

            # layer 2
            pass_a(tb1[1][:], ab[0])
            pass_b(2, tb2[1][:], ab[0], wsb["wl2"], wsb["wr2"], wsb["bl2"],
                   hT[1], None, None, None)

    nc.compile()
    return nc


# --------------------------------------------------------------------------
# Entry point
# --------------------------------------------------------------------------

def _make_in_maps(cfg: Cfg, host, weights):
    iota = np.broadcast_to(np.arange(P, dtype=np.float32), (P, P)).astype(BF16)
    ident = np.eye(P, dtype=np.float32)
    ones = np.ones((1, P), np.float32)
    maps = []
    for i in range(cfg.n_cores):
        m = dict(
            xp1=host["xp1"], xp2=host["xp2"],
            xtloc=host["xT"][i],
            idx=host["idx"][i],
            dstloc=host["dstloc"][i],
            dinv=host["dinv"][i],
            iota=iota, ident=ident, identb=ident.astype(BF16), ones=ones,
        )
        m.update(weights)
        maps.append(m)
    return maps


def _pad_w(w, outp):
    w = np.asarray(w, np.float32)
    if w.shape[-1] < outp:
        pad = np.zeros(w.shape[:-1] + (outp - w.shape[-1],), np.float32)
        w = np.concatenate([w, pad], axis=-1)
    return w


def _ensure_ntff_hook():
    import types

    try:
        from antenv.axon_hooks import get_axon_ntff_profile_hook  # noqa: F401
        return
    except ImportError:
        pass
    try:
        import antenv
        from trn_agent_boot.trn_boot import _ntff_profile_via_ctypes
    except ImportError:
        return
    hook = _ntff_profile_via_ctypes("/opt/axon/libaxon_pjrt.so")
    mod = types.ModuleType("antenv.axon_hooks")
    mod.get_axon_ntff_profile_hook = lambda: hook
    mod.set_axon_ntff_profile_hook = lambda h: None
    sys.modules["antenv.axon_hooks"] = mod
    antenv.axon_hooks = mod


def run(cfg: Cfg, inputs, trace=False):
    import concourse.bass_utils as bu
    from concourse.bass_utils import run_bass_kernel_spmd

    if trace:
        _ensure_ntff_hook()
        bu.upload_artifacts = lambda d: str(d)

    x = np.asarray(inputs["x"], np.float32)
    ei = np.asarray(inputs["edge_index"])
    src = ei[0].astype(np.int64)
    dst = ei[1].astype(np.int64)

    host = prep_host(cfg, x, src, dst)
    weights = dict(
        wl0=np.asarray(inputs["Wl0"], np.float32),
        wr0=np.asarray(inputs["Wr0"], np.float32),
        wl1=np.asarray(inputs["Wl1"], np.float32),
        wr1=np.asarray(inputs["Wr1"], np.float32),
        wl2=_pad_w(inputs["Wl2"], cfg.out_p),
        wr2=_pad_w(inputs["Wr2"], cfg.out_p),
        bl0=np.asarray(inputs["bl0"], np.float32).reshape(1, -1),
        bl1=np.asarray(inputs["bl1"], np.float32).reshape(1, -1),
        bl2=_pad_w(np.asarray(inputs["bl2"], np.float32).reshape(1, -1),
                   cfg.out_p),
    )

    nc = build_program(cfg, host["plan"])
    in_maps = _make_in_maps(cfg, host, weights)
    res = run_bass_kernel_spmd(nc, in_maps, core_ids=list(range(cfg.n_cores)),
                               trace=trace)

    out_full = np.empty((cfg.n_nodes, cfg.out_f), np.float32)
    node_of_pos = host["node_of_pos"]
    for i in range(cfg.n_cores):
        o = res.results[i]["out"]
        pos = np.arange(i * cfg.slots_core, (i + 1) * cfg.slots_core)
        nodes = node_of_pos[pos]
        valid = nodes >= 0
        out_full[nodes[valid]] = o[valid][:, :cfg.out_f]
    return out_full, res


def kernel(**inputs) -> np.ndarray:
    trace = os.environ.get("GNN_TRACE", "0") == "1"
    out, _ = run(FULL_CFG, inputs, trace=trace)
    return out


# revision 30
# speedup vs baseline: 1.0740x; 1.0241x over previous
"""Distributed GraphSAGE (3-layer, mean aggregation) on 8 Trainium2 NeuronCores.

Strategy (ClusterGCN-style node partitioning):
  - Nodes are load-balance-packed into 128-slot "blocks" (degree-aware snake
    packing), 49 blocks per core -> each core owns 6272 node slots.
  - The gather table (h, bf16) is split into two parts: part1 = every core's
    blocks 0..24, part2 = blocks 25..48.  Each part is AllGathered as soon as
    its blocks finish, so collectives overlap compute, and each part stays
    below the int16 index range of dma_gather.
  - Each layer runs two passes over its 49 destination blocks: pass A
    aggregates part1-sourced edges (partial sums parked in SBUF), pass B
    aggregates part2-sourced edges, combines, and runs the dense epilogue.
    Pass A only depends on part1's AllGather, pass B on part2's -> the Pool
    engine (descriptor generation, the critical resource) never idles at
    layer boundaries.
  - Per block, per pass: dma_gather (custom SWDGE ucode, <=1024 descriptors
    per call, round-robin over 4 SWDGE queues, per-block-trimmed counts)
    pulls source rows (bf16); the Vector engine builds a one-hot
    (dst-slot == iota) bf16 matrix; the Tensor engine accumulates
    agg[j, f] += onehot[e, j]^T @ msgs[e, f] in PSUM = segment_sum over the
    block's 128 destination slots.
  - mean = agg * deg_inv (per-partition scale, Scalar engine), transpose on
    the Tensor engine, dense h = mean @ Wl + x @ Wr + bl (+ ReLU, f32).
Weights are replicated; the permuted x (bf16, in part layout) is replicated
as the layer-0 gather table.
"""

import sys

if "/opt/trn_rl_repo" not in sys.path:
    sys.path.insert(0, "/opt/trn_rl_repo")

import os
import numpy as np
import ml_dtypes

BF16 = ml_dtypes.bfloat16
P = 128


class Cfg:
    def __init__(self, n_nodes, n_edges, in_f, hid, out_f, n_cores, blocks_per_core):
        self.n_nodes = n_nodes
        self.n_edges = n_edges
        self.in_f = in_f
        self.hid = hid
        self.out_f = out_f
        self.out_p = 64
        self.n_cores = n_cores
        self.nb = blocks_per_core
        self.nb1 = (blocks_per_core + 1) // 2       # blocks in part1
        self.nb2 = blocks_per_core - self.nb1       # blocks in part2
        self.nblk = n_cores * blocks_per_core
        self.slots_core = blocks_per_core * P
        self.slots_tot = self.nblk * P
        self.p1_rows = n_cores * self.nb1 * P
        self.p2_rows = n_cores * self.nb2 * P
        assert self.p1_rows < 2**15 and self.p2_rows < 2**15


FULL_CFG = Cfg(n_nodes=50000, n_edges=800000, in_f=128, hid=128, out_f=47,
               n_cores=8, blocks_per_core=49)

MAXD = 1024  # dma_gather descriptor-ring limit per call


def _ru16(x):
    return (int(x) + 15) // 16 * 16


class EdgePlan:
    """Static per-(block, part) gather layout, identical across cores."""

    def __init__(self, caps):
        # caps[b][part] = padded edge count (multiple of 16)
        self.caps = caps
        nblocks = len(caps)
        self.tsub = [[(c + P - 1) // P for c in caps[b]] for b in range(nblocks)]
        self.dcol_off = []   # dstloc column offset per block
        self.icol_off = []   # wrapped-idx column offset per (block, part)
        d = 0
        i = 0
        for b in range(nblocks):
            self.dcol_off.append(d)
            offs = []
            for part in range(2):
                offs.append(i)
                i += caps[b][part] // 16
            self.icol_off.append(offs)
            d += self.tsub[b][0] + self.tsub[b][1]
        self.dcols = d
        self.icols = i

    def chunks(self, b, part):
        """Yield (t0_local, num_idxs, icol) chunks of <=MAXD descriptors."""
        cap = self.caps[b][part]
        icol = self.icol_off[b][part]
        done = 0
        while done < cap:
            n = min(MAXD, cap - done)
            yield done // P, n, icol + done // 16
            done += n


def prep_host(cfg: Cfg, x, src, dst):
    N, NBLK = cfg.n_nodes, cfg.nblk
    deg = np.bincount(dst, minlength=N).astype(np.int64)
    deg_inv = (1.0 / np.maximum(deg, 1)).astype(np.float32)

    # Degree-sorted snake deal -> balanced block loads, <=128 slots/block.
    order = np.argsort(-deg, kind="stable")
    k = np.arange(N)
    s, r = k // NBLK, k % NBLK
    blk_for_rank = np.where(s % 2 == 0, r, NBLK - 1 - r)
    blk_of_node = np.empty(N, np.int64)
    blk_of_node[order] = blk_for_rank
    slot_of_node = np.empty(N, np.int64)
    slot_of_node[order] = s
    assert slot_of_node.max() < P

    pos_of_node = blk_of_node * P + slot_of_node
    node_of_pos = np.full(cfg.slots_tot, -1, np.int64)
    node_of_pos[pos_of_node] = np.arange(N)

    # Source position in part-table coordinates
    n_core = blk_of_node // cfg.nb      # owning core
    n_bl = blk_of_node % cfg.nb         # block-local index
    n_part = (n_bl >= cfg.nb1).astype(np.int64)
    n_blp = np.where(n_part == 0, n_bl, n_bl - cfg.nb1)
    nbp = np.array([cfg.nb1, cfg.nb2])
    n_pos_p = (n_core * nbp[n_part] + n_blp) * P + slot_of_node

    # Edge keys: (dst block, src part)
    e_blk = blk_of_node[dst]
    e_part = n_part[src]
    key = e_blk * 2 + e_part
    e_order = np.argsort(key, kind="stable")
    cnt = np.bincount(key, minlength=NBLK * 2).reshape(NBLK, 2)
    # per-(local block, part) cap = max over cores, rounded to 16
    cnt_c = cnt.reshape(cfg.n_cores, cfg.nb, 2)
    caps = [[_ru16(cnt_c[:, b, part].max()) for part in range(2)]
            for b in range(cfg.nb)]
    plan = EdgePlan(caps)

    starts = np.zeros(NBLK * 2 + 1, np.int64)
    np.cumsum(cnt.reshape(-1), out=starts[1:])
    rank = np.arange(cfg.n_edges) - starts[key[e_order]]

    es, ed = src[e_order], dst[e_order]
    eb, ep = e_blk[e_order], e_part[e_order]
    e_core = eb // cfg.nb
    e_bl = eb % cfg.nb

    dstloc = np.full((cfg.n_cores, P, plan.dcols), float(P), np.float32)
    idxw = np.zeros((cfg.n_cores, 16, plan.icols), np.int16)

    dcol_off = np.array(plan.dcol_off)
    t0_of_part = np.array([[0, plan.tsub[b][0]] for b in range(cfg.nb)])
    icol_off = np.array(plan.icol_off)

    # dstloc[core, p, dcol_off[b] + t0_of_part + rank//P] (col), row rank%P
    dcol = dcol_off[e_bl] + t0_of_part[e_bl, ep] + rank // P
    dstloc[e_core, rank % P, dcol] = slot_of_node[ed].astype(np.float32)
    # wrapped idx: within (b, part) region, idx i -> [i%16, off + i//16]
    icol = icol_off[e_bl, ep] + rank // 16
    idxw[e_core, rank % 16, icol] = n_pos_p[es].astype(np.int16)

    dstloc = dstloc.astype(BF16)
    idx_arr = np.ascontiguousarray(np.tile(idxw, (1, 8, 1)))

    dinv_slot = np.ones((cfg.n_cores, P, cfg.nb), np.float32)
    vpos = np.nonzero(node_of_pos >= 0)[0]
    vcore = vpos // cfg.slots_core
    vloc = vpos % cfg.slots_core
    dinv_slot[vcore, vloc % P, vloc // P] = deg_inv[node_of_pos[vpos]]

    # x in part layout (bf16 gather tables) + per-core transposed slice (f32)
    x_perm = np.zeros((cfg.slots_tot, cfg.in_f), np.float32)
    x_perm[pos_of_node] = x
    xp1 = np.zeros((cfg.p1_rows, cfg.in_f), np.float32)
    xp2 = np.zeros((cfg.p2_rows, cfg.in_f), np.float32)
    m1 = n_part[np.arange(N)] == 0
    xp1[n_pos_p[m1]] = x[m1]
    xp2[n_pos_p[~m1]] = x[~m1]
    xT = np.ascontiguousarray(
        x_perm.reshape(cfg.n_cores, cfg.slots_core, cfg.in_f).transpose(0, 2, 1))

    return dict(plan=plan, idx=idx_arr, dstloc=dstloc, dinv=dinv_slot,
                xp1=xp1.astype(BF16), xp2=xp2.astype(BF16), xT=xT,
                node_of_pos=node_of_pos)


# --------------------------------------------------------------------------
# Device program
# --------------------------------------------------------------------------

def build_program(cfg: Cfg, plan: EdgePlan):
    import concourse.bacc as bacc
    import concourse.tile as tile
    from concourse import bass, mybir
    from concourse.library_config import mlp

    f32 = mybir.dt.float32
    bf16 = mybir.dt.bfloat16
    i16 = mybir.dt.int16
    AF = mybir.ActivationFunctionType
    OP = mybir.AluOpType

    NB, NB1, S = cfg.nb, cfg.nb1, cfg.slots_core
    HID, OUTP = cfg.hid, cfg.out_p
    NQ = 4
    TMAX = max(max(plan.tsub[b]) for b in range(NB))

    nc = bacc.Bacc("TRN2", target_bir_lowering=False, debug=False,
                   enable_asserts=False, num_devices=cfg.n_cores,
                   num_swdge_queues=NQ)

    xp1_d = nc.dram_tensor("xp1", [cfg.p1_rows, cfg.in_f], bf16,
                           kind="ExternalInput").ap()
    xp2_d = nc.dram_tensor("xp2", [cfg.p2_rows, cfg.in_f], bf16,
                           kind="ExternalInput").ap()
    xtloc = nc.dram_tensor("xtloc", [cfg.in_f, S], f32,
                           kind="ExternalInput").ap()
    idx_d = nc.dram_tensor("idx", [P, plan.icols], i16,
                           kind="ExternalInput").ap()
    dstloc_d = nc.dram_tensor("dstloc", [P, plan.dcols], bf16,
                              kind="ExternalInput").ap()
    dinv_d = nc.dram_tensor("dinv", [P, NB], f32, kind="ExternalInput").ap()
    w_d = {}
    for name, shp, dt_ in [("wl0", [cfg.in_f, HID], f32),
                           ("wr0", [cfg.in_f, HID], f32),
                           ("wl1", [HID, HID], f32), ("wr1", [HID, HID], f32),
                           ("wl2", [HID, OUTP], f32), ("wr2", [HID, OUTP], f32),
                           ("bl0", [1, HID], f32), ("bl1", [1, HID], f32),
                           ("bl2", [1, OUTP], f32),
                           ("iota", [P, P], bf16), ("ident", [P, P], f32),
                           ("identb", [P, P], bf16), ("ones", [1, P], f32)]:
        w_d[name] = nc.dram_tensor(name, shp, dt_, kind="ExternalInput").ap()
    out_d = nc.dram_tensor("out", [S, OUTP], f32, kind="ExternalOutput").ap()

    qctr = [0]

    with tile.TileContext(nc) as tc:
        with tc.tile_critical():
            nc.gpsimd.load_library(mlp)
        with (
            tc.tile_pool(name="const", bufs=1) as cp,
            tc.tile_pool(name="hT", bufs=2) as hTp,
            tc.tile_pool(name="msgs", bufs=6) as mp,
            tc.tile_pool(name="oh", bufs=4) as ohp,
            tc.tile_pool(name="sb", bufs=3) as sbp,
            tc.tile_pool(name="pagg", bufs=2, space="PSUM") as pagg,
            tc.tile_pool(name="ptr", bufs=2, space="PSUM") as ptr,
            tc.tile_pool(name="pd", bufs=2, space="PSUM") as pd,
            tc.tile_pool(name="dram", bufs=2, space="DRAM") as dp,
        ):
            def load_const(ap_, shp, dtype=f32, tag=None):
                t = cp.tile(shp, dtype, tag=tag or ap_.tensor.name)
                nc.sync.dma_start(out=t[:], in_=ap_)
                return t

            idx_sb = load_const(idx_d, [P, plan.icols], i16)
            dst_sb = load_const(dstloc_d, [P, plan.dcols], bf16)
            dinv_sb = load_const(dinv_d, [P, NB])
            iota_sb = load_const(w_d["iota"], [P, P], bf16)
            ident_sb = load_const(w_d["ident"], [P, P])
            identb_sb = load_const(w_d["identb"], [P, P], bf16)
            ones_sb = load_const(w_d["ones"], [1, P])
            wsb = {k: load_const(w_d[k], list(w_d[k].shape))
                   for k in ("wl0", "wr0", "wl1", "wr1", "wl2", "wr2",
                             "bl0", "bl1", "bl2")}

            hT_x = hTp.tile([P, S], f32, tag="hT", name="hT_x")
            nc.sync.dma_start(out=hT_x[:], in_=xtloc)

            iota_b = iota_sb[:].rearrange("p (a c) -> p a c", a=1)

            def gather_part(table_ap, b, part):
                tsub = plan.tsub[b][part]
                if tsub == 0:
                    return None
                msgs = mp.tile([P, TMAX * HID], bf16, tag="msgs")
                cap = plan.caps[b][part]
                if cap % P:
                    # zero the final subtile before gathering into it: rows
                    # the gather won't write must stay finite (0 * stale-NaN
                    # would poison the aggregation)
                    nc.vector.memset(
                        msgs[:, (tsub - 1) * HID:tsub * HID], 0)
                for t0, nidx, icol in plan.chunks(b, part):
                    nt = (nidx + P - 1) // P
                    nc.gpsimd.dma_gather(
                        out_ap=msgs[:, t0 * HID:(t0 + nt) * HID].rearrange(
                            "p (t c) -> p t c", c=HID),
                        in_ap=table_ap,
                        idxs_ap=idx_sb[:, icol:icol + nidx // 16],
                        num_idxs=nidx,
                        num_idxs_reg=nidx,
                        elem_size=HID,
                        queue_num=qctr[0] % NQ,
                    )
                    qctr[0] += 1
                return msgs

            def agg_matmuls(msgs, b, part, agg_ps):
                tsub = plan.tsub[b][part]
                dcol = plan.dcol_off[b] + (0 if part == 0 else plan.tsub[b][0])
                O = ohp.tile([P, TMAX * P], bf16, tag="oh")
                nc.vector.tensor_tensor(
                    out=O[:, :tsub * P].rearrange("p (t c) -> p t c", t=tsub),
                    in0=dst_sb[:, dcol:dcol + tsub].to_broadcast([P, tsub, P]),
                    in1=iota_b.to_broadcast([P, tsub, P]),
                    op=OP.is_equal,
                )
                for t in range(tsub):
                    nc.tensor.matmul(
                        out=agg_ps[:],
                        lhsT=O[:, t * P:(t + 1) * P],
                        rhs=msgs[:, t * HID:(t + 1) * HID],
                        start=(t == 0), stop=(t == tsub - 1),
                    )

            def pass_a(table_ap, aggbuf):
                for b in range(NB):
                    msgs = gather_part(table_ap, b, 0)
                    agg_ps = pagg.tile([P, HID], f32, tag="agg")
                    if msgs is None:
                        nc.vector.memset(aggbuf[:, b * P:(b + 1) * P], 0)
                        continue
                    agg_matmuls(msgs, b, 0, agg_ps)
                    nc.vector.tensor_copy(out=aggbuf[:, b * P:(b + 1) * P],
                                          in_=agg_ps[:])

            def pass_b(li, table_ap, aggbuf, Wl, Wr, bl, hT_prev, hT_cur,
                       bounce1, bounce2):
                K = OUTP if li == 2 else HID
                for b in range(NB):
                    msgs = gather_part(table_ap, b, 1)
                    mean_pre = sbp.tile([P, HID], f32, tag="mean_pre")
                    if msgs is not None:
                        agg_ps = pagg.tile([P, HID], f32, tag="agg")
                        agg_matmuls(msgs, b, 1, agg_ps)
                        nc.vector.tensor_tensor(
                            out=mean_pre[:], in0=aggbuf[:, b * P:(b + 1) * P],
                            in1=agg_ps[:], op=OP.add)
                    else:
                        nc.vector.tensor_copy(
                            out=mean_pre[:], in_=aggbuf[:, b * P:(b + 1) * P])
                    mean_sb = sbp.tile([P, HID], f32, tag="mean")
                    nc.scalar.activation(mean_sb[:], mean_pre[:], AF.Copy,
                                         scale=dinv_sb[:, b:b + 1])
                    ptr_t = ptr.tile([P, P], f32, tag="tr")
                    nc.tensor.transpose(ptr_t[:], mean_sb[:], ident_sb[:])
                    meanT = sbp.tile([P, P], f32, tag="meanT")
                    nc.vector.tensor_copy(out=meanT[:], in_=ptr_t[:])

                    dens = pd.tile([P, K], f32, tag="dense")
                    nc.tensor.matmul(out=dens[:], lhsT=meanT[:], rhs=Wl[:],
                                     start=True, stop=False)
                    nc.tensor.matmul(out=dens[:],
                                     lhsT=hT_prev[:, b * P:(b + 1) * P],
                                     rhs=Wr[:], start=False, stop=False)
                    nc.tensor.matmul(out=dens[:], lhsT=ones_sb[:], rhs=bl[:],
                                     start=False, stop=True)
                    if li < 2:
                        h_b = sbp.tile([P, HID], bf16, tag="h_b")
                        nc.scalar.activation(h_b[:], dens[:], AF.Relu)
                        if b < NB1:
                            nc.sync.dma_start(
                                out=bounce1[b * P:(b + 1) * P, :], in_=h_b[:])
                        else:
                            bb = b - NB1
                            nc.sync.dma_start(
                                out=bounce2[bb * P:(bb + 1) * P, :], in_=h_b[:])
                        ptr_t2 = ptr.tile([P, P], bf16, tag="trb", bufs=1)
                        nc.tensor.transpose(ptr_t2[:], h_b[:], identb_sb[:])
                        nc.vector.tensor_copy(out=hT_cur[:, b * P:(b + 1) * P],
                                              in_=ptr_t2[:])
                    else:
                        out_b = sbp.tile([P, OUTP], f32, tag="out_b")
                        nc.scalar.activation(out_b[:], dens[:], AF.Copy)
                        nc.sync.dma_start(out=out_d[b * P:(b + 1) * P, :],
                                          in_=out_b[:])

            def allgather(bounce, full):
                nc.gpsimd.collective_compute(
                    "AllGather", OP.bypass,
                    replica_groups=[list(range(cfg.n_cores))],
                    ins=[bounce.opt()], outs=[full.opt()],
                )

            # DRAM staging (double buffered across layers)
            tb1 = [dp.tile([cfg.p1_rows, HID], bf16, tag="t1",
                           addr_space="Shared", name=f"t1_{i}") for i in range(2)]
            tb2 = [dp.tile([cfg.p2_rows, HID], bf16, tag="t2",
                           addr_space="Shared", name=f"t2_{i}") for i in range(2)]
            bn1 = [dp.tile([cfg.nb1 * P, HID], bf16, tag="b1", name=f"b1_{i}")
                   for i in range(2)]
            bn2 = [dp.tile([cfg.nb2 * P, HID], bf16, tag="b2", name=f"b2_{i}")
                   for i in range(2)]

            hT = [hTp.tile([P, S], f32, tag="hT", name=f"hT{i}")
                  for i in range(2)]
            ab = [hTp.tile([P, S], f32, tag="aggbuf", name=f"ab{i}")
                  for i in range(2)]

            # layer 0 (tables = x parts, already resident)
            pass_a(xp1_d, ab[0])
            pass_b(0, xp2_d, ab[0], wsb["wl0"], wsb["wr0"], wsb["bl0"],
                   hT_x, hT[0], bn1[0][:], bn2[0][:])
            allgather(bn1[0], tb1[0])
            allgather(bn2[0], tb2[0])

            # layer 1
            pass_a(tb1[0][:], ab[1])
            pass_b(1, tb2[0][:], ab[1], wsb["wl1"], wsb["wr1"], wsb["bl1"],
                   hT[0], hT[1], bn1[1][:], bn2[1][:])
            allgather(bn1[1], tb1[1])
            allgather(bn2[1], tb2[1])

            # layer 2
            pass_a(tb1[1][:], ab[0])
            pass_b(2, tb2[1][:], ab[0], wsb["wl2"], wsb["wr2"], wsb["bl2"],
                   hT[1], None, None, None)

    nc.compile()
    return nc


# --------------------------------------------------------------------------
# Entry point
# --------------------------------------------------------------------------

def _make_in_maps(cfg: Cfg, host, weights):
    iota = np.broadcast_to(np.arange(P, dtype=np.float32), (P, P)).astype(BF16)
    ident = np.eye(P, dtype=np.float32)
    ones = np.ones((1, P), np.float32)
    maps = []
    for i in range(cfg.n_cores):
        m = dict(
            xp1=host["xp1"], xp2=host["xp2"],
            xtloc=host["xT"][i],
            idx=host["idx"][i],
            dstloc=host["dstloc"][i],
            dinv=host["dinv"][i],
            iota=iota, ident=ident, identb=ident.astype(BF16), ones=ones,
        )
        m.update(weights)
        maps.append(m)
    return maps


def _pad_w(w, outp):
    w = np.asarray(w, np.float32)
    if w.shape[-1] < outp:
        pad = np.zeros(w.shape[:-1] + (outp - w.shape[-1],), np.float32)
        w = np.concatenate([w, pad], axis=-1)
    return w


def _ensure_ntff_hook():
    import types

    try:
        from antenv.axon_hooks import get_axon_ntff_profile_hook  # noqa: F401
        return
    except ImportError:
        pass
    try:
        import antenv
        from trn_agent_boot.trn_boot import _ntff_profile_via_ctypes
    except ImportError:
        return
    hook = _ntff_profile_via_ctypes("/opt/axon/libaxon_pjrt.so")
    mod = types.ModuleType("antenv.axon_hooks")
    mod.get_axon_ntff_profile_hook = lambda: hook
    mod.set_axon_ntff_profile_hook = lambda h: None
    sys.modules["antenv.axon_hooks"] = mod
    antenv.axon_hooks = mod


def run(cfg: Cfg, inputs, trace=False):
    import concourse.bass_utils as bu
    from concourse.bass_utils import run_bass_kernel_spmd

    if trace:
        _ensure_ntff_hook()
        bu.upload_artifacts = lambda d: str(d)

    x = np.asarray(inputs["x"], np.float32)
    ei = np.asarray(inputs["edge_index"])
    src = ei[0].astype(np.int64)
    dst = ei[1].astype(np.int64)

    host = prep_host(cfg, x, src, dst)
    weights = dict(
        wl0=np.asarray(inputs["Wl0"], np.float32),
        wr0=np.asarray(inputs["Wr0"], np.float32),
        wl1=np.asarray(inputs["Wl1"], np.float32),
        wr1=np.asarray(inputs["Wr1"], np.float32),
        wl2=_pad_w(inputs["Wl2"], cfg.out_p),
        wr2=_pad_w(inputs["Wr2"], cfg.out_p),
        bl0=np.asarray(inputs["bl0"], np.float32).reshape(1, -1),
        bl1=np.asarray(inputs["bl1"], np.float32).reshape(1, -1),
        bl2=_pad_w(np.asarray(inputs["bl2"], np.float32).reshape(1, -1),
                   cfg.out_p),
    )

    nc = build_program(cfg, host["plan"])
    in_maps = _make_in_maps(cfg, host, weights)
    res = run_bass_kernel_spmd(nc, in_maps, core_ids=list(range(cfg.n_cores)),
                               trace=trace)

    out_full = np.empty((cfg.n_nodes, cfg.out_f), np.float32)
    node_of_pos = host["node_of_pos"]
    for i in range(cfg.n_cores):
        o = res.results[i]["out"]
        pos = np.arange(i * cfg.slots_core, (i + 1) * cfg.slots_core)
        nodes = node_of_pos[pos]
        valid = nodes >= 0
        out_full[nodes[valid]] = o[valid][:, :cfg.out_f]
    return out_full, res


def kernel(**inputs) -> np.ndarray:
    trace = os.environ.get("GNN_TRACE", "0") == "1"
    out, _ = run(FULL_CFG, inputs, trace=trace)
    return out


# revision 31
# speedup vs baseline: 1.1098x; 1.0333x over previous
"""Distributed GraphSAGE (3-layer, mean aggregation) on 8 Trainium2 NeuronCores.

Strategy (ClusterGCN-style node partitioning):
  - Nodes are load-balance-packed into 128-slot "blocks" (degree-aware snake
    packing), 49 blocks per core -> each core owns 6272 node slots.
  - The gather table (h, bf16) is split into two parts: part1 = every core's
    blocks 0..24, part2 = blocks 25..48.  Each part is AllGathered as soon as
    its blocks finish, so collectives overlap compute, and each part stays
    below the int16 index range of dma_gather.
  - Each layer runs two passes over its 49 destination blocks: pass A
    aggregates part1-sourced edges (partial sums parked in SBUF), pass B
    aggregates part2-sourced edges, combines, and runs the dense epilogue.
    Pass A only depends on part1's AllGather, pass B on part2's -> the Pool
    engine (descriptor generation, the critical resource) never idles at
    layer boundaries.
  - Per block, per pass: dma_gather (custom SWDGE ucode, <=1024 descriptors
    per call, round-robin over 4 SWDGE queues, per-block-trimmed counts)
    pulls source rows (bf16); the Vector engine builds a one-hot
    (dst-slot == iota) bf16 matrix; the Tensor engine accumulates
    agg[j, f] += onehot[e, j]^T @ msgs[e, f] in PSUM = segment_sum over the
    block's 128 destination slots.
  - mean = agg * deg_inv (per-partition scale, Scalar engine), transpose on
    the Tensor engine, dense h = mean @ Wl + x @ Wr + bl (+ ReLU, f32).
Weights are replicated; the permuted x (bf16, in part layout) is replicated
as the layer-0 gather table.
"""

import sys

if "/opt/trn_rl_repo" not in sys.path:
    sys.path.insert(0, "/opt/trn_rl_repo")

import os
import numpy as np
import ml_dtypes

BF16 = ml_dtypes.bfloat16
P = 128


class Cfg:
    def __init__(self, n_nodes, n_edges, in_f, hid, out_f, n_cores, blocks_per_core):
        self.n_nodes = n_nodes
        self.n_edges = n_edges
        self.in_f = in_f
        self.hid = hid
        self.out_f = out_f
        self.out_p = 64
        self.n_cores = n_cores
        self.nb = blocks_per_core
        self.nb1 = (blocks_per_core + 1) // 2       # blocks in part1
        self.nb2 = blocks_per_core - self.nb1       # blocks in part2
        self.nblk = n_cores * blocks_per_core
        self.slots_core = blocks_per_core * P
        self.slots_tot = self.nblk * P
        self.p1_rows = n_cores * self.nb1 * P
        self.p2_rows = n_cores * self.nb2 * P
        assert self.p1_rows < 2**15 and self.p2_rows < 2**15


FULL_CFG = Cfg(n_nodes=50000, n_edges=800000, in_f=128, hid=128, out_f=47,
               n_cores=8, blocks_per_core=49)

MAXD = 1024  # dma_gather descriptor-ring limit per call


def _ru16(x):
    return (int(x) + 15) // 16 * 16


class EdgePlan:
    """Static per-(block, part) gather layout, identical across cores."""

    def __init__(self, caps):
        # caps[b][part] = padded edge count (multiple of 16)
        self.caps = caps
        nblocks = len(caps)
        self.tsub = [[(c + P - 1) // P for c in caps[b]] for b in range(nblocks)]
        self.dcol_off = []   # dstloc column offset per block
        self.icol_off = []   # wrapped-idx column offset per (block, part)
        d = 0
        i = 0
        for b in range(nblocks):
            self.dcol_off.append(d)
            offs = []
            for part in range(2):
                offs.append(i)
                i += caps[b][part] // 16
            self.icol_off.append(offs)
            d += self.tsub[b][0] + self.tsub[b][1]
        self.dcols = d
        self.icols = i

    def chunks(self, b, part):
        """Yield (t0_local, num_idxs, icol) chunks of <=MAXD descriptors."""
        cap = self.caps[b][part]
        icol = self.icol_off[b][part]
        done = 0
        while done < cap:
            n = min(MAXD, cap - done)
            yield done // P, n, icol + done // 16
            done += n


def prep_host(cfg: Cfg, x, src, dst):
    N, NBLK = cfg.n_nodes, cfg.nblk
    deg = np.bincount(dst, minlength=N).astype(np.int64)
    deg_inv = (1.0 / np.maximum(deg, 1)).astype(np.float32)

    # Degree-sorted snake deal -> balanced block loads, <=128 slots/block.
    order = np.argsort(-deg, kind="stable")
    k = np.arange(N)
    s, r = k // NBLK, k % NBLK
    blk_for_rank = np.where(s % 2 == 0, r, NBLK - 1 - r)
    blk_of_node = np.empty(N, np.int64)
    blk_of_node[order] = blk_for_rank
    slot_of_node = np.empty(N, np.int64)
    slot_of_node[order] = s
    assert slot_of_node.max() < P

    pos_of_node = blk_of_node * P + slot_of_node
    node_of_pos = np.full(cfg.slots_tot, -1, np.int64)
    node_of_pos[pos_of_node] = np.arange(N)

    # Source position in part-table coordinates
    n_core = blk_of_node // cfg.nb      # owning core
    n_bl = blk_of_node % cfg.nb         # block-local index
    n_part = (n_bl >= cfg.nb1).astype(np.int64)
    n_blp = np.where(n_part == 0, n_bl, n_bl - cfg.nb1)
    nbp = np.array([cfg.nb1, cfg.nb2])
    n_pos_p = (n_core * nbp[n_part] + n_blp) * P + slot_of_node

    # Edge keys: (dst block, src part)
    e_blk = blk_of_node[dst]
    e_part = n_part[src]
    key = e_blk * 2 + e_part
    e_order = np.argsort(key, kind="stable")
    cnt = np.bincount(key, minlength=NBLK * 2).reshape(NBLK, 2)
    # per-(local block, part) cap = max over cores, rounded to 16
    cnt_c = cnt.reshape(cfg.n_cores, cfg.nb, 2)
    caps = [[_ru16(cnt_c[:, b, part].max()) for part in range(2)]
            for b in range(cfg.nb)]
    plan = EdgePlan(caps)

    starts = np.zeros(NBLK * 2 + 1, np.int64)
    np.cumsum(cnt.reshape(-1), out=starts[1:])
    rank = np.arange(cfg.n_edges) - starts[key[e_order]]

    es, ed = src[e_order], dst[e_order]
    eb, ep = e_blk[e_order], e_part[e_order]
    e_core = eb // cfg.nb
    e_bl = eb % cfg.nb

    dstloc = np.full((cfg.n_cores, P, plan.dcols), float(P), np.float32)
    idxw = np.zeros((cfg.n_cores, 16, plan.icols), np.int16)

    dcol_off = np.array(plan.dcol_off)
    t0_of_part = np.array([[0, plan.tsub[b][0]] for b in range(cfg.nb)])
    icol_off = np.array(plan.icol_off)

    # dstloc[core, p, dcol_off[b] + t0_of_part + rank//P] (col), row rank%P
    dcol = dcol_off[e_bl] + t0_of_part[e_bl, ep] + rank // P
    dstloc[e_core, rank % P, dcol] = slot_of_node[ed].astype(np.float32)
    # wrapped idx: within (b, part) region, idx i -> [i%16, off + i//16]
    icol = icol_off[e_bl, ep] + rank // 16
    idxw[e_core, rank % 16, icol] = n_pos_p[es].astype(np.int16)

    dstloc = dstloc.astype(BF16)
    idx_arr = np.ascontiguousarray(np.tile(idxw, (1, 8, 1)))

    dinv_slot = np.ones((cfg.n_cores, P, cfg.nb), np.float32)
    vpos = np.nonzero(node_of_pos >= 0)[0]
    vcore = vpos // cfg.slots_core
    vloc = vpos % cfg.slots_core
    dinv_slot[vcore, vloc % P, vloc // P] = deg_inv[node_of_pos[vpos]]

    # x in part layout (bf16 gather tables) + per-core transposed slice (f32)
    x_perm = np.zeros((cfg.slots_tot, cfg.in_f), np.float32)
    x_perm[pos_of_node] = x
    xp1 = np.zeros((cfg.p1_rows, cfg.in_f), np.float32)
    xp2 = np.zeros((cfg.p2_rows, cfg.in_f), np.float32)
    m1 = n_part[np.arange(N)] == 0
    xp1[n_pos_p[m1]] = x[m1]
    xp2[n_pos_p[~m1]] = x[~m1]
    xT = np.ascontiguousarray(
        x_perm.reshape(cfg.n_cores, cfg.slots_core, cfg.in_f).transpose(0, 2, 1))

    return dict(plan=plan, idx=idx_arr, dstloc=dstloc, dinv=dinv_slot,
                xp1=xp1.astype(BF16), xp2=xp2.astype(BF16), xT=xT,
                node_of_pos=node_of_pos)


# --------------------------------------------------------------------------
# Device program
# --------------------------------------------------------------------------

def build_program(cfg: Cfg, plan: EdgePlan):
    import concourse.bacc as bacc
    import concourse.tile as tile
    from concourse import bass, mybir
    from concourse.library_config import mlp

    f32 = mybir.dt.float32
    bf16 = mybir.dt.bfloat16
    i16 = mybir.dt.int16
    AF = mybir.ActivationFunctionType
    OP = mybir.AluOpType

    NB, NB1, S = cfg.nb, cfg.nb1, cfg.slots_core
    HID, OUTP = cfg.hid, cfg.out_p
    NQ = 4
    TMAX = max(max(plan.tsub[b]) for b in range(NB))

    nc = bacc.Bacc("TRN2", target_bir_lowering=False, debug=False,
                   enable_asserts=False, num_devices=cfg.n_cores,
                   num_swdge_queues=NQ)

    xp1_d = nc.dram_tensor("xp1", [cfg.p1_rows, cfg.in_f], bf16,
                           kind="ExternalInput").ap()
    xp2_d = nc.dram_tensor("xp2", [cfg.p2_rows, cfg.in_f], bf16,
                           kind="ExternalInput").ap()
    xtloc = nc.dram_tensor("xtloc", [cfg.in_f, S], f32,
                           kind="ExternalInput").ap()
    idx_d = nc.dram_tensor("idx", [P, plan.icols], i16,
                           kind="ExternalInput").ap()
    dstloc_d = nc.dram_tensor("dstloc", [P, plan.dcols], bf16,
                              kind="ExternalInput").ap()
    dinv_d = nc.dram_tensor("dinv", [P, NB], f32, kind="ExternalInput").ap()
    w_d = {}
    for name, shp, dt_ in [("wl0", [cfg.in_f, HID], f32),
                           ("wr0", [cfg.in_f, HID], f32),
                           ("wl1", [HID, HID], f32), ("wr1", [HID, HID], f32),
                           ("wl2", [HID, OUTP], f32), ("wr2", [HID, OUTP], f32),
                           ("bl0", [1, HID], f32), ("bl1", [1, HID], f32),
                           ("bl2", [1, OUTP], f32),
                           ("iota", [P, P], bf16), ("ident", [P, P], f32),
                           ("identb", [P, P], bf16), ("ones", [1, P], f32)]:
        w_d[name] = nc.dram_tensor(name, shp, dt_, kind="ExternalInput").ap()
    out_d = nc.dram_tensor("out", [S, OUTP], f32, kind="ExternalOutput").ap()

    qctr = [0]

    with tile.TileContext(nc) as tc:
        with tc.tile_critical():
            nc.gpsimd.load_library(mlp)
        with (
            tc.tile_pool(name="const", bufs=1) as cp,
            tc.tile_pool(name="hT", bufs=2) as hTp,
            tc.tile_pool(name="msgs", bufs=8) as mp,
            tc.tile_pool(name="oh", bufs=6) as ohp,
            tc.tile_pool(name="sb", bufs=3) as sbp,
            tc.tile_pool(name="pagg", bufs=2, space="PSUM") as pagg,
            tc.tile_pool(name="ptr", bufs=2, space="PSUM") as ptr,
            tc.tile_pool(name="pd", bufs=2, space="PSUM") as pd,
            tc.tile_pool(name="dram", bufs=2, space="DRAM") as dp,
        ):
            def load_const(ap_, shp, dtype=f32, tag=None):
                t = cp.tile(shp, dtype, tag=tag or ap_.tensor.name)
                nc.sync.dma_start(out=t[:], in_=ap_)
                return t

            idx_sb = load_const(idx_d, [P, plan.icols], i16)
            dst_sb = load_const(dstloc_d, [P, plan.dcols], bf16)
            dinv_sb = load_const(dinv_d, [P, NB])
            iota_sb = load_const(w_d["iota"], [P, P], bf16)
            ident_sb = load_const(w_d["ident"], [P, P])
            identb_sb = load_const(w_d["identb"], [P, P], bf16)
            ones_sb = load_const(w_d["ones"], [1, P])
            wsb = {k: load_const(w_d[k], list(w_d[k].shape))
                   for k in ("wl0", "wr0", "wl1", "wr1", "wl2", "wr2",
                             "bl0", "bl1", "bl2")}

            hT_x = hTp.tile([P, S], f32, tag="hT", name="hT_x")
            nc.sync.dma_start(out=hT_x[:], in_=xtloc)

            iota_b = iota_sb[:].rearrange("p (a c) -> p a c", a=1)

            def gather_part(table_ap, b, part):
                tsub = plan.tsub[b][part]
                if tsub == 0:
                    return None
                msgs = mp.tile([P, TMAX * HID], bf16, tag="msgs")
                cap = plan.caps[b][part]
                if cap % P:
                    # zero the final subtile before gathering into it: rows
                    # the gather won't write must stay finite (0 * stale-NaN
                    # would poison the aggregation)
                    nc.vector.memset(
                        msgs[:, (tsub - 1) * HID:tsub * HID], 0)
                for t0, nidx, icol in plan.chunks(b, part):
                    nt = (nidx + P - 1) // P
                    nc.gpsimd.dma_gather(
                        out_ap=msgs[:, t0 * HID:(t0 + nt) * HID].rearrange(
                            "p (t c) -> p t c", c=HID),
                        in_ap=table_ap,
                        idxs_ap=idx_sb[:, icol:icol + nidx // 16],
                        num_idxs=nidx,
                        num_idxs_reg=nidx,
                        elem_size=HID,
                        queue_num=qctr[0] % NQ,
                    )
                    qctr[0] += 1
                return msgs

            def agg_matmuls(msgs, b, part, agg_ps):
                tsub = plan.tsub[b][part]
                dcol = plan.dcol_off[b] + (0 if part == 0 else plan.tsub[b][0])
                O = ohp.tile([P, TMAX * P], bf16, tag="oh")
                nc.vector.tensor_tensor(
                    out=O[:, :tsub * P].rearrange("p (t c) -> p t c", t=tsub),
                    in0=dst_sb[:, dcol:dcol + tsub].to_broadcast([P, tsub, P]),
                    in1=iota_b.to_broadcast([P, tsub, P]),
                    op=OP.is_equal,
                )
                for t in range(tsub):
                    nc.tensor.matmul(
                        out=agg_ps[:],
                        lhsT=O[:, t * P:(t + 1) * P],
                        rhs=msgs[:, t * HID:(t + 1) * HID],
                        start=(t == 0), stop=(t == tsub - 1),
                    )

            def pass_a(table_ap, aggbuf):
                for b in range(NB):
                    msgs = gather_part(table_ap, b, 0)
                    agg_ps = pagg.tile([P, HID], f32, tag="agg")
                    if msgs is None:
                        nc.vector.memset(aggbuf[:, b * P:(b + 1) * P], 0)
                        continue
                    agg_matmuls(msgs, b, 0, agg_ps)
                    nc.vector.tensor_copy(out=aggbuf[:, b * P:(b + 1) * P],
                                          in_=agg_ps[:])

            def pass_b(li, table_ap, aggbuf, Wl, Wr, bl, hT_prev, hT_cur,
                       bounce1, bounce2):
                K = OUTP if li == 2 else HID
                for b in range(NB):
                    msgs = gather_part(table_ap, b, 1)
                    mean_pre = sbp.tile([P, HID], f32, tag="mean_pre")
                    if msgs is not None:
                        agg_ps = pagg.tile([P, HID], f32, tag="agg")
                        agg_matmuls(msgs, b, 1, agg_ps)
                        nc.vector.tensor_tensor(
                            out=mean_pre[:], in0=aggbuf[:, b * P:(b + 1) * P],
                            in1=agg_ps[:], op=OP.add)
                    else:
                        nc.vector.tensor_copy(
                            out=mean_pre[:], in_=aggbuf[:, b * P:(b + 1) * P])
                    mean_sb = sbp.tile([P, HID], f32, tag="mean")
                    nc.scalar.activation(mean_sb[:], mean_pre[:], AF.Copy,
                                         scale=dinv_sb[:, b:b + 1])
                    ptr_t = ptr.tile([P, P], f32, tag="tr")
                    nc.tensor.transpose(ptr_t[:], mean_sb[:], ident_sb[:])
                    meanT = sbp.tile([P, P], f32, tag="meanT")
                    nc.vector.tensor_copy(out=meanT[:], in_=ptr_t[:])

                    dens = pd.tile([P, K], f32, tag="dense")
                    nc.tensor.matmul(out=dens[:], lhsT=meanT[:], rhs=Wl[:],
                                     start=True, stop=False)
                    nc.tensor.matmul(out=dens[:],
                                     lhsT=hT_prev[:, b * P:(b + 1) * P],
                                     rhs=Wr[:], start=False, stop=False)
                    nc.tensor.matmul(out=dens[:], lhsT=ones_sb[:], rhs=bl[:],
                                     start=False, stop=True)
                    if li < 2:
                        h_b = sbp.tile([P, HID], bf16, tag="h_b")
                        nc.scalar.activation(h_b[:], dens[:], AF.Relu)
                        if b < NB1:
                            nc.sync.dma_start(
                                out=bounce1[b * P:(b + 1) * P, :], in_=h_b[:])
                        else:
                            bb = b - NB1
                            nc.sync.dma_start(
                                out=bounce2[bb * P:(bb + 1) * P, :], in_=h_b[:])
                        ptr_t2 = ptr.tile([P, P], bf16, tag="trb", bufs=1)
                        nc.tensor.transpose(ptr_t2[:], h_b[:], identb_sb[:])
                        nc.vector.tensor_copy(out=hT_cur[:, b * P:(b + 1) * P],
                                              in_=ptr_t2[:])
                    else:
                        out_b = sbp.tile([P, OUTP], f32, tag="out_b")
                        nc.scalar.activation(out_b[:], dens[:], AF.Copy)
                        nc.sync.dma_start(out=out_d[b * P:(b + 1) * P, :],
                                          in_=out_b[:])

            def allgather(bounce, full):
                nc.gpsimd.collective_compute(
                    "AllGather", OP.bypass,
                    replica_groups=[list(range(cfg.n_cores))],
                    ins=[bounce.opt()], outs=[full.opt()],
                )

            # DRAM staging (double buffered across layers)
            tb1 = [dp.tile([cfg.p1_rows, HID], bf16, tag="t1",
                           addr_space="Shared", name=f"t1_{i}") for i in range(2)]
            tb2 = [dp.tile([cfg.p2_rows, HID], bf16, tag="t2",
                           addr_space="Shared", name=f"t2_{i}") for i in range(2)]
            bn1 = [dp.tile([cfg.nb1 * P, HID], bf16, tag="b1", name=f"b1_{i}")
                   for i in range(2)]
            bn2 = [dp.tile([cfg.nb2 * P, HID], bf16, tag="b2", name=f"b2_{i}")
                   for i in range(2)]

            hT = [hTp.tile([P, S], f32, tag="hT", name=f"hT{i}")
                  for i in range(2)]
            ab = [hTp.tile([P, S], f32, tag="aggbuf", name=f"ab{i}")
                  for i in range(2)]

            # layer 0 (tables = x parts, already resident)
            pass_a(xp1_d, ab[0])
            pass_b(0, xp2_d, ab[0], wsb["wl0"], wsb["wr0"], wsb["bl0"],
                   hT_x, hT[0], bn1[0][:], bn2[0][:])
            allgather(bn1[0], tb1[0])
            allgather(bn2[0], tb2[0])

            # layer 1
            pass_a(tb1[0][:], ab[1])
            pass_b(1, tb2[0][:], ab[1], wsb["wl1"], wsb["wr1"], wsb["bl1"],
                   hT[0], hT[1], bn1[1][:], bn2[1][:])
            allgather(bn1[1], tb1[1])
            allgather(bn2[1], tb2[1])

            # layer 2
            pass_a(tb1[1][:], ab[0])
            pass_b(2, tb2[1][:], ab[0], wsb["wl2"], wsb["wr2"], wsb["bl2"],
                   hT[1], None, None, None)

    nc.compile()
    return nc


# --------------------------------------------------------------------------
# Entry point
# --------------------------------------------------------------------------

def _make_in_maps(cfg: Cfg, host, weights):
    iota = np.broadcast_to(np.arange(P, dtype=np.float32), (P, P)).astype(BF16)
    ident = np.eye(P, dtype=np.float32)
    ones = np.ones((1, P), np.float32)
    maps = []
    for i in range(cfg.n_cores):
        m = dict(
            xp1=host["xp1"], xp2=host["xp2"],
            xtloc=host["xT"][i],
            idx=host["idx"][i],
            dstloc=host["dstloc"][i],
            dinv=host["dinv"][i],
            iota=iota, ident=ident, identb=ident.astype(BF16), ones=ones,
        )
        m.update(weights)
        maps.append(m)
    return maps


def _pad_w(w, outp):
    w = np.asarray(w, np.float32)
    if w.shape[-1] < outp:
        pad = np.zeros(w.shape[:-1] + (outp - w.shape[-1],), np.float32)
        w = np.concatenate([w, pad], axis=-1)
    return w


def _ensure_ntff_hook():
    import types

    try:
        from antenv.axon_hooks import get_axon_ntff_profile_hook  # noqa: F401
        return
    except ImportError:
        pass
    try:
        import antenv
        from trn_agent_boot.trn_boot import _ntff_profile_via_ctypes
    except ImportError:
        return
    hook = _ntff_profile_via_ctypes("/opt/axon/libaxon_pjrt.so")
    mod = types.ModuleType("antenv.axon_hooks")
    mod.get_axon_ntff_profile_hook = lambda: hook
    mod.set_axon_ntff_profile_hook = lambda h: None
    sys.modules["antenv.axon_hooks"] = mod
    antenv.axon_hooks = mod


def run(cfg: Cfg, inputs, trace=False):
    import concourse.bass_utils as bu
    from concourse.bass_utils import run_bass_kernel_spmd

    if trace:
        _ensure_ntff_hook()
        bu.upload_artifacts = lambda d: str(d)

    x = np.asarray(inputs["x"], np.float32)
    ei = np.asarray(inputs["edge_index"])
    src = ei[0].astype(np.int64)
    dst = ei[1].astype(np.int64)

    host = prep_host(cfg, x, src, dst)
    weights = dict(
        wl0=np.asarray(inputs["Wl0"], np.float32),
        wr0=np.asarray(inputs["Wr0"], np.float32),
        wl1=np.asarray(inputs["Wl1"], np.float32),
        wr1=np.asarray(inputs["Wr1"], np.float32),
        wl2=_pad_w(inputs["Wl2"], cfg.out_p),
        wr2=_pad_w(inputs["Wr2"], cfg.out_p),
        bl0=np.asarray(inputs["bl0"], np.float32).reshape(1, -1),
        bl1=np.asarray(inputs["bl1"], np.float32).reshape(1, -1),
        bl2=_pad_w(np.asarray(inputs["bl2"], np.float32).reshape(1, -1),
                   cfg.out_p),
    )

    nc = build_program(cfg, host["plan"])
    in_maps = _make_in_maps(cfg, host, weights)
    res = run_bass_kernel_spmd(nc, in_maps, core_ids=list(range(cfg.n_cores)),
                               trace=trace)

    out_full = np.empty((cfg.n_nodes, cfg.out_f), np.float32)
    node_of_pos = host["node_of_pos"]
    for i in range(cfg.n_cores):
        o = res.results[i]["out"]
        pos = np.arange(i * cfg.slots_core, (i + 1) * cfg.slots_core)
        nodes = node_of_pos[pos]
        valid = nodes >= 0
        out_full[nodes[valid]] = o[valid][:, :cfg.out_f]
    return out_full, res


def kernel(**inputs) -> np.ndarray:
    trace = os.environ.get("GNN_TRACE", "0") == "1"
    out, _ = run(FULL_CFG, inputs, trace=trace)
    return out
